# revision 38
# baseline (speedup 1.0000x reference)
"""Trainium2 Bass kernel for nn_HOR_16870631539538 (dense_transformer).

Module (per batch item b, C=64 channels, hw=4096 spatial):
  stage 1: p = x_low^T conv outputs attention [hw,hw], softmax over axis n,
           e = p_sm @ v + x_low
  stage 2: t = conv_e(e) @ xl2_sp  (64x64), softmax over c, out = x_mid @ t_sm

Sharding: 8 cores = 4 batch items x 2 halves of the softmax-column dim (m).
Each core computes exp/softmax for its m-half only (the expensive part).
Key algebraic trick: downstream only needs G = e @ xl2_sp  (64x65 incl. the
ones-row for the e_conv bias term), which is linear in the m-partial e, so the
cross-core combine is ONE 16KB AllReduce of G instead of 1MiB of e.

Layouts: conv outputs channel-major [c, n]; attention computed transposed
pT[m, n] so softmax axis n is the free dim (ACT exp with fused accum_out).
The 1/denominator is folded into the small v matrix, never touching the slab.
The x_low residual is injected into the PSUM e-accumulators by one extra
matmul per block against a 0.5*I stationary (no staging DMA).

Dtypes: fp16 inputs/convs/pT (stage-2 softmax is sensitive to absolute noise
in t, bf16 fails); exp slab + vs bf16 (|p| reaches ~45: exp(p) and 1/d need
an 8-bit exponent; no max-subtraction required); G / t path fp32.

The PE streams ~1 moving column/cycle only when the contraction depth is
128; K=64 matmuls run ~2.2x slower per column (measured).  The big pT
matmuls therefore zero-pad xlowT / xl_hi partitions 64-127 (the only
memsets); small convs stay K=64 reading [0:C] slices.
"""

import numpy as np

import concourse.bacc as bacc
import concourse.bass as bass
import concourse.mybir as mybir
import concourse.tile as tile
from concourse.bass_utils import run_bass_kernel_spmd

dt = mybir.dt
AF = mybir.ActivationFunctionType
ALU = mybir.AluOpType

N_CORES = 8
C = 64
HW = 4096
MH = HW // 2           # per-core m-half (2048)
NCHUNK = MH // 128     # 16 m-chunks of 128 rows
NB = HW // 512         # 8 n-blocks of 512

DT_IN = dt.float16     # input tensors + conv weights + pT operands
DT_SLAB = dt.bfloat16  # exp slab + vs (bf16: range-safe for exp w/o max)
DT_G = dt.float32      # e_sp / xl2sp operands for the G matmuls
DT_OUT = dt.float16    # tsm / xmidT operands for the output matmul

USE_COLLECTIVE = True
TRUNC = 99   # debug: 1=convs 3=+mloop 36=+G 4=+stage 99=full

_CACHE = {}


def build():
    nc = bacc.Bacc("TRN2", target_bir_lowering=False, debug=False,
                   num_devices=N_CORES)

    def din(name, shape, dtype=dt.float32):
        return nc.dram_tensor(name, shape, dtype, kind="ExternalInput").ap()

    xin = din("xin", [C, HW], DT_IN)      # x[b] channel-major, fp16
    xlat = din("xlat", [C, HW], DT_IN)    # x_latter[b], fp16
    # blob16: cols 0-319 = wlT whT wvT wlatT wmT (rows 0-63);
    #         cols 320-383 = 0.5*I residual stationary (rows 0-127)
    blob16 = din("blob16", [128, 6 * C], DT_IN)
    # blob32: cols 0-63 = weT (rows 0-63) + be row (row 64);
    #         cols 64-127 = identity at rows 0-63 AND at rows 64-127 (the
    #         high copy feeds transposes of PSUM partitions 64-127);
    #         cols 128-132 = bl bh bv blat bm columns (rows 0-63)
    blob32 = din("blob32", [128, 2 * C + 5], dt.float32)
    outp = nc.dram_tensor("outp", [C, MH], dt.float32,
                          kind="ExternalOutput").ap()

    with tile.TileContext(nc) as tc:
        _body(nc, tc, locals())
    nc.compile()
    return nc


def _body(nc, tc, io):
    ts = bass.ts

    const = tc.alloc_tile_pool(name="const", bufs=1)
    big = tc.alloc_tile_pool(name="big", bufs=1)
    slabp = tc.alloc_tile_pool(name="slabp", bufs=2)
    mm = tc.alloc_tile_pool(name="mm", bufs=2, space="PSUM")
    acc = tc.alloc_tile_pool(name="acc", bufs=1, space="PSUM")
    dram = tc.alloc_tile_pool(name="dram", bufs=1, space="DRAM")

    # ---- constants: two packed DMAs ----
    blob16 = const.tile([128, 6 * C], DT_IN, tag="blob16")
    blob32 = const.tile([128, 2 * C + 5], dt.float32, tag="blob32")
    nc.sync.dma_start(blob16[:], io["blob16"])
    nc.sync.dma_start(blob32[:], io["blob32"])
    wlT = blob16[0:C, 0 * C:1 * C]
    whT = blob16[0:C, 1 * C:2 * C]
    wvT = blob16[0:C, 2 * C:3 * C]
    wlatT = blob16[0:C, 3 * C:4 * C]
    wmT = blob16[0:C, 4 * C:5 * C]
    halfid = blob16[:, 5 * C:6 * C]
    weT = blob32[0:C, 0:C]
    be_row = blob32[64:65, 0:C]
    idf32 = blob32[0:C, C:2 * C]
    idf32hi = blob32[64:128, C:2 * C]
    bl = blob32[0:C, 2 * C + 0:2 * C + 1]
    bh = blob32[0:C, 2 * C + 1:2 * C + 2]
    bv = blob32[0:C, 2 * C + 2:2 * C + 3]
    blat = blob32[0:C, 2 * C + 3:2 * C + 4]
    bm = blob32[0:C, 2 * C + 4:2 * C + 5]

    # ---- pT operand tiles need zero partitions 64-127 (K=128 matmuls);
    # memsets run on gpsimd before anything else ----
    xlowT = big.tile([128, HW], DT_IN, tag="xlowT")    # conv_low(x), full n
    xl_hi = big.tile([128, MH], DT_IN, tag="xl_hi")    # conv_high(xlat) m-half
    nc.gpsimd.memset(xl_hi[C:128, :], 0.0)
    nc.gpsimd.memset(xlowT[C:128, 0:MH], 0.0)
    nc.gpsimd.memset(xlowT[C:128, MH:HW], 0.0)

    # ---- inputs: split by partition range as well as columns -- DMA cost
    # is dominated by per-partition descriptor dispatch on one queue, so
    # [0:32]/[32:64] halves on separate queues halve the latency ----
    xin = big.tile([C, HW], DT_IN, tag="xin")
    xlat = big.tile([C, HW], DT_IN, tag="xlat")
    for lo, hi in ((0, 32), (32, 64)):
        nc.sync.dma_start(xlat[lo:hi, 0:MH], io["xlat"][lo:hi, 0:MH])
        nc.sync.dma_start(xin[lo:hi, 0:MH], io["xin"][lo:hi, 0:MH])
        nc.sync.dma_start(xin[lo:hi, MH:HW], io["xin"][lo:hi, MH:HW])
        nc.sync.dma_start(xlat[lo:hi, MH:HW], io["xlat"][lo:hi, MH:HW])

    v_s = big.tile([C, MH], dt.float32, tag="v_s")     # conv_value(x) m-half
    xl2 = big.tile([C, HW], DT_G, tag="xl2")           # conv_latter(xlat) full
    xmidT = big.tile([C, MH], DT_OUT, tag="xmidT")     # conv_mid(x) n-half
    sacc = big.tile([C, 4], dt.float32, tag="sacc")    # xl2 row-sum parts

    CH = 1024

    # ---- conv helper: out_sbuf[c, n] = W @ x + b  (K=64, [0:C] slices) ----
    def conv_chunk(dst, wT, src, bias, j, accum=None, ji=0, evict="vector"):
        pt = mm.tile([C, CH], dt.float32, tag="mmt")
        for k in range(0, CH, 512):
            nc.tensor.matmul(pt[:, k:k + 512], wT,
                             src[0:C, j + k:j + k + 512],
                             start=True, stop=True)
        if accum is not None:
            nc.vector.tensor_scalar(dst[0:C, j:j + CH], pt[:], bias, 0.0,
                                    ALU.add, ALU.add,
                                    accum_out=accum[:, ji:ji + 1])
        elif evict == "scalar":
            nc.scalar.activation(dst[0:C, j:j + CH], pt[:], AF.Identity,
                                 bias=bias)
        else:
            nc.vector.tensor_scalar(dst[0:C, j:j + CH], pt[:], bias, None,
                                    ALU.add)

    # Only the chunks gating the first pT passes run before the loop (on
    # ACT so the DVE queue can't delay them); everything else is slotted
    # into the loop body to keep the PE queue open for iter-0 work.
    v_sp = big.tile([128, NCHUNK * C], dt.float32, tag="v_sp")

    conv_chunk(xl_hi, whT, xlat, bh, 0 * CH, evict="scalar")
    conv_chunk(xlowT, wlT, xin, bl, 0 * CH, evict="scalar")
    conv_chunk(xlowT, wlT, xin, bl, 1 * CH, evict="scalar")

    def v_group(g):
        # transpose v: [64, 128] tiles -> 8 tiles [128, 64] of v_sp
        pt = mm.tile([128, 512], dt.float32, tag="mmt")
        for q in range(8):
            i = g + q
            nc.tensor.transpose(pt[:, q * 64:(q + 1) * 64],
                                v_s[:, ts(i, 128)], idf32)
        nc.vector.tensor_copy(v_sp[:, g * 64:(g + 8) * 64], pt[:])

    s_col = big.tile([C, 1], dt.float32, tag="s_col")
    xl2sp = big.tile([128, 32 * C], DT_G, tag="xl2sp")
    gs_stage = big.tile([65, C], dt.float32, tag="gs_stage")

    def xl2sp_group(g):
        # transpose xl2 [64, HW] -> 8 tiles [128, 64] of xl2sp
        pt = mm.tile([128, 512], DT_G, tag="mmt")
        for q in range(8):
            i = g + q
            nc.tensor.transpose(pt[:, q * 64:(q + 1) * 64],
                                xl2[:, ts(i, 128)], idf32)
        nc.vector.tensor_copy(xl2sp[:, g * 64:(g + 8) * 64], pt[:])

    def s_row_stage():
        # s[d] = sum_n xl2[d, n]; transpose to a row, scaled 0.5
        nc.vector.reduce_sum(s_col[:], sacc[:], axis=mybir.AxisListType.X)
        spt = mm.tile([128, 512], dt.float32, tag="mmt")
        nc.tensor.transpose(spt[0:1, 0:64], s_col[:], idf32)
        nc.vector.tensor_scalar(gs_stage[64:65, :], spt[0:1, 0:64], 0.5,
                                None, ALU.mult)

    # work slotted into the m-loop's spare engine cycles.  mid_extras run
    # between pT passes 1 and 2 (iter 0 needs xlowT c2/c3 just in time);
    # end_extras run after the iteration's e-acc matmuls.
    mid_extras = {
        0: [lambda: conv_chunk(xlowT, wlT, xin, bl, 2 * CH),
            lambda: conv_chunk(xlowT, wlT, xin, bl, 3 * CH)],
    }
    # post_extras run after pass 3 but BEFORE the e-acc matmuls: e-acc
    # waits on vs <- v_sp <- the v transposes, and the PE runs in issue
    # order, so the transposes must be queued ahead of e-acc.
    post_extras = {
        0: [lambda: conv_chunk(v_s, wvT, xin, bv, 0 * CH),
            lambda: conv_chunk(v_s, wvT, xin, bv, 1 * CH),
            lambda: v_group(0), lambda: v_group(8)],
    }
    end_extras = {
        1: [lambda: conv_chunk(xl_hi, whT, xlat, bh, 1 * CH)],
        2: [lambda: conv_chunk(xl2, wlatT, xlat, blat, 0 * CH,
                               accum=sacc, ji=0)],
        3: [lambda: conv_chunk(xl2, wlatT, xlat, blat, 1 * CH,
                               accum=sacc, ji=1)],
        4: [lambda: conv_chunk(xl2, wlatT, xlat, blat, 2 * CH,
                               accum=sacc, ji=2), lambda: xl2sp_group(0)],
        5: [lambda: conv_chunk(xl2, wlatT, xlat, blat, 3 * CH,
                               accum=sacc, ji=3), lambda: xl2sp_group(8)],
        6: [lambda: conv_chunk(xmidT, wmT, xin, bm, 0 * CH),
            lambda: xl2sp_group(16)],
        7: [lambda: conv_chunk(xmidT, wmT, xin, bm, 1 * CH),
            lambda: xl2sp_group(24)],
        8: [lambda: s_row_stage()],
    }

    def finish(src_ap):
        osb = big.tile([C, MH], dt.float32, tag="osb")
        nc.gpsimd.memset(osb[:], 0.0)
        nc.vector.tensor_copy(osb[:, 0:src_ap.shape[1]], src_ap)
        nc.sync.dma_start(io["outp"], osb[:])
        for p in (dram, acc, mm, slabp, big, const):
            p.release()

    # ---- eT accumulators: 4 psum tiles [128, 512]; partition half p holds
    # n-block 2k+p. ----
    accs = [acc.tile([128, 512], dt.float32, tag=f"acc{k}", name=f"acc{k}")
            for k in range(4)]

    # ---- stage-1 m-loop ----
    for i in range(NCHUNK):
        slab = slabp.tile([128, HW], DT_SLAB, tag="slab")
        dacc = slabp.tile([128, 4], dt.float32, tag="dacc")
        for j in range(4):  # pT sub-passes of [128, 1024]
            pt = mm.tile([128, 1024], dt.float32, tag="mmt")
            for k in range(2):
                nc.tensor.matmul(pt[:, k * 512:(k + 1) * 512],
                                 xl_hi[:, ts(i, 128)],
                                 xlowT[:, j * 1024 + k * 512:
                                       j * 1024 + (k + 1) * 512],
                                 start=True, stop=True)
            nc.scalar.activation(slab[:, j * 1024:(j + 1) * 1024], pt[:],
                                 AF.Exp, accum_out=dacc[:, j:j + 1])
            if j == 1:
                for x in mid_extras.get(i, ()):
                    x()
        for x in post_extras.get(i, ()):
            x()
        dsum = slabp.tile([128, 1], dt.float32, tag="dsum")
        nc.vector.reduce_sum(dsum[:], dacc[:], axis=mybir.AxisListType.X)
        rec = slabp.tile([128, 1], dt.float32, tag="rec")
        nc.vector.reciprocal(rec[:], dsum[:])
        vs = slabp.tile([128, C], DT_SLAB, tag="vs")
        nc.vector.tensor_scalar(vs[:], v_sp[:, ts(i, C)], rec[:], None,
                                ALU.mult)
        first = i == 0
        for k in range(4):
            for p in range(2):
                blk = 2 * k + p
                nc.tensor.matmul(accs[k][p * 64:(p + 1) * 64, :], vs[:],
                                 slab[:, ts(blk, 512)], start=first,
                                 stop=False, skip_group_check=True)
        for x in end_extras.get(i, ()):
            x()

    # close each accumulator with the 0.5*x_low residual: one matmul per
    # (k, p) against the 0.5*I stationary, reusing the zero-padded xlowT.
    # (0.5: the AllReduce over the core pair sums two copies of the residual
    # and bias-row terms, so each core contributes half.)
    for k in range(4):
        for p in range(2):
            blk = 2 * k + p
            nc.tensor.matmul(accs[k][p * 64:(p + 1) * 64, :], halfid,
                             xlowT[:, ts(blk, 512)], start=False,
                             stop=True, skip_group_check=True)

    # ---- tail: per-block evict e, pipelined with the odd-half partition-
    # move DMAs (fine-split: descriptor dispatch dominates, so 16-desc
    # pieces across many queues), transposes and the G matmuls ----
    e_h = big.tile([128, 4 * 512], DT_G, tag="e_h")
    e_h2 = big.tile([C, 4 * 512], DT_G, tag="e_h2")
    for k in range(4):
        nc.vector.tensor_copy(e_h[:, ts(k, 512)], accs[k][:])
        for lo, hi in ((0, 16), (16, 32), (32, 48), (48, 64)):
            nc.sync.dma_start(e_h2[lo:hi, ts(k, 512)],
                              e_h[64 + lo:64 + hi, ts(k, 512)])

    if TRUNC == 3:
        return finish(e_h[0:64, :])

    # groups ordered so even blocks (straight from e_h) go first while the
    # e_h2 partition-move DMAs complete.
    e_sp = big.tile([128, 32 * C], DT_G, tag="e_sp")
    gps = acc.tile([128, 512], dt.float32, tag="acc0")
    G = gps[0:64, 0:64]
    groups = [(0, 2), (4, 6), (1, 3), (5, 7)]
    n_done = 0
    for gi, blocks in enumerate(groups):
        pt = mm.tile([128, 512], DT_G, tag="mmt")
        tids = []
        for q, blk in enumerate([b for b in blocks for _ in range(4)]):
            sl = q % 4                 # 128-col slice within block
            t_idx = blk * 4 + sl
            tids.append(t_idx)
            k, p = blk // 2, blk % 2
            src = e_h if p == 0 else e_h2
            nc.tensor.transpose(
                pt[:, q * 64:(q + 1) * 64],
                src[0:64, k * 512 + sl * 128:k * 512 + (sl + 1) * 128],
                idf32)
        for half in range(2):
            t0 = tids[half * 4]
            nc.vector.tensor_copy(e_sp[:, t0 * 64:(t0 + 4) * 64],
                                  pt[:, half * 256:(half + 1) * 256])
        for t_idx in tids:
            nc.tensor.matmul(G, e_sp[:, ts(t_idx, C)], xl2sp[:, ts(t_idx, C)],
                             start=(n_done == 0), stop=(n_done == 31),
                             skip_group_check=True)
            n_done += 1

    if TRUNC == 36:
        gtmp = big.tile([C, C], dt.float32, tag="gtmp")
        nc.vector.tensor_copy(gtmp[:], G)
        return finish(gtmp[:])

    nc.vector.tensor_copy(gs_stage[0:64, :], G)

    if TRUNC == 4:
        return finish(gs_stage[0:64, :])

    # ---- AllReduce G over core pairs ----
    gs_red = big.tile([65, C], dt.float32, tag="gs_red")
    if USE_COLLECTIVE:
        g_in = dram.tile([65, C], dt.float32, tag="g_in")
        g_out = dram.tile([65, C], dt.float32, tag="g_out")
        # partition-split DMAs: staging cost is per-partition descriptor
        # dispatch, so 4 queues in parallel cut it 4x
        for lo, hi in ((0, 17), (17, 33), (33, 49), (49, 65)):
            nc.sync.dma_start(g_in[lo:hi, :], gs_stage[lo:hi, :])
        nc.gpsimd.collective_compute(
            "AllReduce", ALU.add,
            ins=[g_in.opt()], outs=[g_out.opt()],
            replica_groups=[[0, 1], [2, 3], [4, 5], [6, 7]],
        )
        for lo, hi in ((0, 17), (17, 33), (33, 49), (49, 65)):
            nc.sync.dma_start(gs_red[lo:hi, :], g_out[lo:hi, :])
    else:
        nc.vector.tensor_copy(gs_red[:], gs_stage[:])

    # ---- t = We @ G + be x s  (fp32 matmuls) ----
    tps = mm.tile([128, 512], dt.float32, tag="mmt")
    t_ps = tps[0:64, 0:64]
    nc.tensor.matmul(t_ps, weT, gs_red[0:64, :], start=True, stop=False,
                     skip_group_check=True)
    nc.tensor.matmul(t_ps, be_row, gs_red[64:65, :], start=False,
                     stop=True, tile_position=(64, 0), skip_group_check=True)
    t_s = big.tile([C, C], dt.float32, tag="t_s")
    nc.vector.tensor_copy(t_s[:], t_ps)

    # ---- softmax over c: transpose -> [d, c], exp w/ max, normalize ----
    tt_ps = mm.tile([128, 512], dt.float32, tag="mmt")
    nc.tensor.transpose(tt_ps[0:64, 0:64], t_s[:], idf32)
    tmax = big.tile([C, 1], dt.float32, tag="tmax")
    nc.vector.reduce_max(tmax[:], tt_ps[0:64, 0:64], axis=mybir.AxisListType.X)
    nmax = big.tile([C, 1], dt.float32, tag="nmax")
    nc.vector.tensor_scalar(nmax[:], tmax[:], -1.0, None, ALU.mult)
    texp = big.tile([C, C], dt.float32, tag="texp")
    tsum = big.tile([C, 1], dt.float32, tag="tsum")
    nc.scalar.activation(texp[:], tt_ps[0:64, 0:64], AF.Exp, bias=nmax[:],
                         accum_out=tsum[:])
    trec = big.tile([C, 1], dt.float32, tag="trec")
    nc.vector.reciprocal(trec[:], tsum[:])
    tsmT = big.tile([C, C], dt.float32, tag="tsmT")
    nc.vector.tensor_scalar(tsmT[:], texp[:], trec[:], None, ALU.mult)

    # transpose back -> tsm[c, d] fp16 for the output matmul
    tb_ps = mm.tile([128, 512], dt.float32, tag="mmt")
    nc.tensor.transpose(tb_ps[0:64, 0:64], tsmT[:], idf32)
    tsm = big.tile([C, C], DT_OUT, tag="tsm")
    nc.vector.tensor_copy(tsm[:], tb_ps[0:64, 0:64])

    # ---- out^T[d, n-half] = tsm^T-pair @ xmidT ----
    # evictions alternate DVE/ACT; output DMA per 1024-col half overlaps
    osb = big.tile([C, MH], dt.float32, tag="osb")
    for k in range(4):
        op = mm.tile([C, 512], dt.float32, tag="mmt")
        nc.tensor.matmul(op[:], tsm[:], xmidT[:, ts(k, 512)],
                         start=True, stop=True)
        if k % 2 == 0:
            nc.vector.tensor_copy(osb[:, ts(k, 512)], op[:])
        else:
            nc.scalar.copy(osb[:, ts(k, 512)], op[:])
            for lo, hi in ((0, 16), (16, 32), (32, 48), (48, 64)):
                nc.sync.dma_start(
                    io["outp"][lo:hi, (k - 1) * 512:(k + 1) * 512],
                    osb[lo:hi, (k - 1) * 512:(k + 1) * 512])

    for p in (dram, acc, mm, slabp, big, const):
        p.release()


def _prep_inputs(x_latter, x, W, b):
    """Build the 8 per-core input maps from full inputs."""
    B = x_latter.shape[0]
    xr = x.reshape(B, C, HW).astype(np.float16)
    xlr = x_latter.reshape(B, C, HW).astype(np.float16)
    wT = {k: np.ascontiguousarray(W[k].T) for k in W}

    blob16 = np.zeros((128, 6 * C), np.float16)
    for i, n in enumerate(["low", "high", "value", "latter", "mid"]):
        blob16[0:C, i * C:(i + 1) * C] = wT[n].astype(np.float16)
    blob16[0:C, 5 * C:6 * C] = (0.5 * np.eye(C)).astype(np.float16)

    blob32 = np.zeros((128, 2 * C + 5), np.float32)
    blob32[0:C, 0:C] = wT["e_conv"]
    blob32[64, 0:C] = b["e_conv"].reshape(C)
    blob32[0:C, C:2 * C] = np.eye(C, dtype=np.float32)
    blob32[C:2 * C, C:2 * C] = np.eye(C, dtype=np.float32)
    for i, n in enumerate(["low", "high", "value", "latter", "mid"]):
        blob32[0:C, 2 * C + i] = b[n].reshape(C)

    maps = []
    for core in range(N_CORES):
        bi, h = core // 2, core % 2
        # roll columns so this core's own m-half sits at columns [0, MH)
        xin_c = np.roll(xr[bi], -h * MH, axis=1) if h else xr[bi]
        xlat_c = np.roll(xlr[bi], -h * MH, axis=1) if h else xlr[bi]
        maps.append({
            "xin": np.ascontiguousarray(xin_c),
            "xlat": np.ascontiguousarray(xlat_c),
            "blob16": blob16,
            "blob32": blob32,
        })
    return maps


def run(inputs, trace=False, trace_cores=None):
    if "nc" not in _CACHE:
        _CACHE["nc"] = build()
    nc = _CACHE["nc"]

    names = ["high", "low", "value", "e_conv", "mid", "latter"]
    W = {n: np.asarray(inputs[f"W_{n}"], dtype=np.float32) for n in names}
    b = {n: np.asarray(inputs[f"b_{n}"], dtype=np.float32).reshape(C, 1)
         for n in names}
    x = np.asarray(inputs["x"], dtype=np.float32)
    x_latter = np.asarray(inputs["x_latter"], dtype=np.float32)
    maps = _prep_inputs(x_latter, x, W, b)

    kw = {}
    if trace:
        kw = dict(trace=True,
                  trace_cores=trace_cores or list(range(N_CORES)))
    res = run_bass_kernel_spmd(nc, maps, core_ids=list(range(N_CORES)), **kw)

    B = x_latter.shape[0]
    out = np.empty((B, C, HW), dtype=np.float32)
    for core in range(N_CORES):
        bi, h = core // 2, core % 2
        out[bi][:, h * MH:(h + 1) * MH] = res.results[core]["outp"]
    H = int(np.sqrt(HW))
    return out.reshape(B, C, H, H), res


def kernel(**inputs):
    out, _ = run(inputs, trace=False)
    return out


# revision 43
# speedup vs baseline: 1.0772x; 1.0772x over previous
"""Trainium2 Bass kernel for nn_HOR_16870631539538 (dense_transformer).

Module (per batch item b, C=64 channels, hw=4096 spatial):
  stage 1: p = x_low^T conv outputs attention [hw,hw], softmax over axis n,
           e = p_sm @ v + x_low
  stage 2: t = conv_e(e) @ xl2_sp  (64x64), softmax over c, out = x_mid @ t_sm

Sharding: 8 cores = 4 batch items x 2 halves of the softmax-column dim (m).
Each core computes exp/softmax for its m-half only (the expensive part).
Key algebraic trick: downstream only needs G = e @ xl2_sp  (64x65 incl. the
ones-row for the e_conv bias term), which is linear in the m-partial e, so the
cross-core combine is ONE 16KB AllReduce of G instead of 1MiB of e.

Layouts: conv outputs channel-major [c, n]; attention computed transposed
pT[m, n] so softmax axis n is the free dim (ACT exp with fused accum_out).
The 1/denominator is folded into the small v matrix, never touching the slab.
The x_low residual is injected into the PSUM e-accumulators by one extra
matmul per block against a 0.5*I stationary (no staging DMA).

Dtypes: fp16 inputs/convs/pT (stage-2 softmax is sensitive to absolute noise
in t, bf16 fails); exp slab + vs bf16 (|p| reaches ~45: exp(p) and 1/d need
an 8-bit exponent; no max-subtraction required); G / t path fp32.

The PE streams ~1 moving column/cycle only when the contraction depth is
128; K=64 matmuls run ~2.2x slower per column (measured).  The big pT
matmuls therefore zero-pad xlowT / xl_hi partitions 64-127 (the only
memsets); small convs stay K=64 reading [0:C] slices.
"""

import numpy as np

import concourse.bacc as bacc
import concourse.bass as bass
import concourse.mybir as mybir
import concourse.tile as tile
from concourse.bass_utils import run_bass_kernel_spmd

dt = mybir.dt
AF = mybir.ActivationFunctionType
ALU = mybir.AluOpType

N_CORES = 8
C = 64
HW = 4096
MH = HW // 2           # per-core m-half (2048)
NCHUNK = MH // 128     # 16 m-chunks of 128 rows
NB = HW // 512         # 8 n-blocks of 512

DT_IN = dt.float16     # input tensors + conv weights + pT operands
DT_SLAB = dt.bfloat16  # exp slab + vs (bf16: range-safe for exp w/o max)
DT_G = dt.float32      # e_sp / xl2sp operands for the G matmuls
DT_OUT = dt.float16    # tsm / xmidT operands for the output matmul

USE_COLLECTIVE = True
TRUNC = 99   # debug: 1=convs 3=+mloop 36=+G 4=+stage 99=full

_CACHE = {}


def build():
    nc = bacc.Bacc("TRN2", target_bir_lowering=False, debug=False,
                   num_devices=N_CORES)

    def din(name, shape, dtype=dt.float32):
        return nc.dram_tensor(name, shape, dtype, kind="ExternalInput").ap()

    xin = din("xin", [C, HW], DT_IN)      # x[b] channel-major, fp16
    xlat = din("xlat", [C, HW], DT_IN)    # x_latter[b], fp16
    # blob16: cols 0-319 = wlT whT wvT wlatT wmT (rows 0-63);
    #         cols 320-383 = 0.5*I residual stationary (rows 0-127)
    blob16 = din("blob16", [128, 6 * C], DT_IN)
    # blob32: cols 0-63 = weT (rows 0-63) + be row (row 64);
    #         cols 64-127 = identity at rows 0-63 AND at rows 64-127 (the
    #         high copy feeds transposes of PSUM partitions 64-127);
    #         cols 128-132 = bl bh bv blat bm columns (rows 0-63)
    blob32 = din("blob32", [128, 2 * C + 5], dt.float32)
    outp = nc.dram_tensor("outp", [C, MH], dt.float32,
                          kind="ExternalOutput").ap()

    with tile.TileContext(nc) as tc:
        _body(nc, tc, locals())
    nc.compile()
    return nc


def _body(nc, tc, io):
    ts = bass.ts

    const = tc.alloc_tile_pool(name="const", bufs=1)
    big = tc.alloc_tile_pool(name="big", bufs=1)
    slabp = tc.alloc_tile_pool(name="slabp", bufs=2)
    mm = tc.alloc_tile_pool(name="mm", bufs=2, space="PSUM")
    acc = tc.alloc_tile_pool(name="acc", bufs=1, space="PSUM")
    dram = tc.alloc_tile_pool(name="dram", bufs=1, space="DRAM")

    # ---- constants: two packed DMAs ----
    blob16 = const.tile([128, 6 * C], DT_IN, tag="blob16")
    blob32 = const.tile([128, 2 * C + 5], dt.float32, tag="blob32")
    nc.sync.dma_start(blob16[:], io["blob16"])
    nc.sync.dma_start(blob32[:], io["blob32"])
    wlT = blob16[0:C, 0 * C:1 * C]
    whT = blob16[0:C, 1 * C:2 * C]
    wvT = blob16[0:C, 2 * C:3 * C]
    wlatT = blob16[0:C, 3 * C:4 * C]
    wmT = blob16[0:C, 4 * C:5 * C]
    halfid = blob16[:, 5 * C:6 * C]
    weT = blob32[0:C, 0:C]
    be_row = blob32[64:65, 0:C]
    idf32 = blob32[0:C, C:2 * C]
    idf32hi = blob32[64:128, C:2 * C]
    bl = blob32[0:C, 2 * C + 0:2 * C + 1]
    bh = blob32[0:C, 2 * C + 1:2 * C + 2]
    bv = blob32[0:C, 2 * C + 2:2 * C + 3]
    blat = blob32[0:C, 2 * C + 3:2 * C + 4]
    bm = blob32[0:C, 2 * C + 4:2 * C + 5]

    # ---- pT operand tiles need zero partitions 64-127 (K=128 matmuls);
    # memsets run on gpsimd before anything else ----
    xlowT = big.tile([128, HW], DT_IN, tag="xlowT")    # conv_low(x), full n
    xl_hi = big.tile([128, MH], DT_IN, tag="xl_hi")    # conv_high(xlat) m-half
    nc.gpsimd.memset(xl_hi[C:128, :], 0.0)
    nc.gpsimd.memset(xlowT[C:128, 0:MH], 0.0)
    nc.gpsimd.memset(xlowT[C:128, MH:HW], 0.0)

    # ---- inputs: split by partition range as well as columns -- DMA cost
    # is dominated by per-partition descriptor dispatch on one queue, so
    # [0:32]/[32:64] halves on separate queues halve the latency ----
    xin = big.tile([C, HW], DT_IN, tag="xin")
    xlat = big.tile([C, HW], DT_IN, tag="xlat")
    # the chunks gating the loop start go first, split 4 ways
    for lo, hi in ((0, 16), (16, 32), (32, 48), (48, 64)):
        nc.sync.dma_start(xlat[lo:hi, 0:MH], io["xlat"][lo:hi, 0:MH])
    for lo, hi in ((0, 16), (16, 32), (32, 48), (48, 64)):
        nc.sync.dma_start(xin[lo:hi, 0:MH], io["xin"][lo:hi, 0:MH])
    for lo, hi in ((0, 32), (32, 64)):
        nc.sync.dma_start(xin[lo:hi, MH:HW], io["xin"][lo:hi, MH:HW])
        nc.sync.dma_start(xlat[lo:hi, MH:HW], io["xlat"][lo:hi, MH:HW])

    v_s = big.tile([C, MH], dt.float32, tag="v_s")     # conv_value(x) m-half
    xl2 = big.tile([C, HW], DT_G, tag="xl2")           # conv_latter(xlat) full
    xmidT = big.tile([128, MH], DT_OUT, tag="xmidT")   # conv_mid(x) n-half
    nc.gpsimd.memset(xmidT[C:128, :], 0.0)             # K=128 out matmuls
    sacc = big.tile([C, 4], dt.float32, tag="sacc")    # xl2 row-sum parts

    CH = 1024

    # ---- conv helper: out_sbuf[c, n] = W @ x + b  (K=64, [0:C] slices) ----
    def conv_chunk(dst, wT, src, bias, j, accum=None, ji=0, evict="vector"):
        pt = mm.tile([C, CH], dt.float32, tag="mmt")
        for k in range(0, CH, 512):
            nc.tensor.matmul(pt[:, k:k + 512], wT,
                             src[0:C, j + k:j + k + 512],
                             start=True, stop=True)
        if accum is not None:
            nc.vector.tensor_scalar(dst[0:C, j:j + CH], pt[:], bias, 0.0,
                                    ALU.add, ALU.add,
                                    accum_out=accum[:, ji:ji + 1])
        elif evict == "scalar":
            nc.scalar.activation(dst[0:C, j:j + CH], pt[:], AF.Identity,
                                 bias=bias)
        else:
            nc.vector.tensor_scalar(dst[0:C, j:j + CH], pt[:], bias, None,
                                    ALU.add)

    # Only the chunks gating the first pT passes run before the loop (on
    # ACT so the DVE queue can't delay them); everything else is slotted
    # into the loop body to keep the PE queue open for iter-0 work.
    v_sp = big.tile([128, NCHUNK * C], dt.float32, tag="v_sp")

    conv_chunk(xl_hi, whT, xlat, bh, 0 * CH, evict="scalar")
    conv_chunk(xlowT, wlT, xin, bl, 0 * CH, evict="scalar")
    conv_chunk(xlowT, wlT, xin, bl, 1 * CH, evict="scalar")

    def v_group(g):
        # transpose v: [64, 128] tiles -> 8 tiles [128, 64] of v_sp
        pt = mm.tile([128, 512], dt.float32, tag="mmt")
        for q in range(8):
            i = g + q
            nc.tensor.transpose(pt[:, q * 64:(q + 1) * 64],
                                v_s[:, ts(i, 128)], idf32)
        nc.vector.tensor_copy(v_sp[:, g * 64:(g + 8) * 64], pt[:])

    s_col = big.tile([C, 1], dt.float32, tag="s_col")
    xl2sp = big.tile([128, 32 * C], DT_G, tag="xl2sp")
    gs_stage = big.tile([65, C], dt.float32, tag="gs_stage")

    def xl2sp_group(g):
        # transpose xl2 [64, HW] -> 8 tiles [128, 64] of xl2sp
        pt = mm.tile([128, 512], DT_G, tag="mmt")
        for q in range(8):
            i = g + q
            nc.tensor.transpose(pt[:, q * 64:(q + 1) * 64],
                                xl2[:, ts(i, 128)], idf32)
        nc.vector.tensor_copy(xl2sp[:, g * 64:(g + 8) * 64], pt[:])

    def s_row_stage():
        # s[d] = sum_n xl2[d, n]; transpose to a row, scaled 0.5
        nc.vector.reduce_sum(s_col[:], sacc[:], axis=mybir.AxisListType.X)
        spt = mm.tile([128, 512], dt.float32, tag="mmt")
        nc.tensor.transpose(spt[0:1, 0:64], s_col[:], idf32)
        nc.vector.tensor_scalar(gs_stage[64:65, :], spt[0:1, 0:64], 0.5,
                                None, ALU.mult)

    # work slotted into the m-loop's spare engine cycles.  mid_extras run
    # between pT passes 1 and 2 (iter 0 needs xlowT c2/c3 just in time);
    # end_extras run after the iteration's e-acc matmuls.
    mid_extras = {
        0: [lambda: conv_chunk(xlowT, wlT, xin, bl, 2 * CH),
            lambda: conv_chunk(xlowT, wlT, xin, bl, 3 * CH)],
    }
    # post_extras run after pass 3 but BEFORE the e-acc matmuls: e-acc
    # waits on vs <- v_sp <- the v transposes, and the PE runs in issue
    # order, so the transposes must be queued ahead of e-acc.
    post_extras = {
        0: [lambda: conv_chunk(v_s, wvT, xin, bv, 0 * CH),
            lambda: conv_chunk(v_s, wvT, xin, bv, 1 * CH),
            lambda: v_group(0), lambda: v_group(8)],
    }
    end_extras = {
        1: [lambda: conv_chunk(xl_hi, whT, xlat, bh, 1 * CH)],
        2: [lambda: conv_chunk(xl2, wlatT, xlat, blat, 0 * CH,
                               accum=sacc, ji=0)],
        3: [lambda: conv_chunk(xl2, wlatT, xlat, blat, 1 * CH,
                               accum=sacc, ji=1)],
        4: [lambda: conv_chunk(xl2, wlatT, xlat, blat, 2 * CH,
                               accum=sacc, ji=2), lambda: xl2sp_group(0)],
        5: [lambda: conv_chunk(xl2, wlatT, xlat, blat, 3 * CH,
                               accum=sacc, ji=3), lambda: xl2sp_group(8)],
        6: [lambda: conv_chunk(xmidT, wmT, xin, bm, 0 * CH),
            lambda: xl2sp_group(16)],
        7: [lambda: conv_chunk(xmidT, wmT, xin, bm, 1 * CH),
            lambda: xl2sp_group(24)],
        8: [lambda: s_row_stage()],
    }

    def finish(src_ap):
        osb = big.tile([C, MH], dt.float32, tag="osb")
        nc.gpsimd.memset(osb[:], 0.0)
        nc.vector.tensor_copy(osb[:, 0:src_ap.shape[1]], src_ap)
        nc.sync.dma_start(io["outp"], osb[:])
        for p in (dram, acc, mm, slabp, big, const):
            p.release()

    # ---- eT accumulators: 4 psum tiles [128, 512]; partition half p holds
    # n-block 2k+p. ----
    accs = [acc.tile([128, 512], dt.float32, tag=f"acc{k}", name=f"acc{k}")
            for k in range(4)]

    # ---- stage-1 m-loop ----
    for i in range(NCHUNK):
        slab = slabp.tile([128, HW], DT_SLAB, tag="slab")
        dacc = slabp.tile([128, 4], dt.float32, tag="dacc")
        for j in range(4):  # pT sub-passes of [128, 1024]
            pt = mm.tile([128, 1024], dt.float32, tag="mmt")
            for k in range(2):
                nc.tensor.matmul(pt[:, k * 512:(k + 1) * 512],
                                 xl_hi[:, ts(i, 128)],
                                 xlowT[:, j * 1024 + k * 512:
                                       j * 1024 + (k + 1) * 512],
                                 start=True, stop=True)
            nc.scalar.activation(slab[:, j * 1024:(j + 1) * 1024], pt[:],
                                 AF.Exp, accum_out=dacc[:, j:j + 1])
            if j == 1:
                for x in mid_extras.get(i, ()):
                    x()
        for x in post_extras.get(i, ()):
            x()
        dsum = slabp.tile([128, 1], dt.float32, tag="dsum")
        nc.vector.reduce_sum(dsum[:], dacc[:], axis=mybir.AxisListType.X)
        rec = slabp.tile([128, 1], dt.float32, tag="rec")
        nc.vector.reciprocal(rec[:], dsum[:])
        vs = slabp.tile([128, C], DT_SLAB, tag="vs")
        nc.vector.tensor_scalar(vs[:], v_sp[:, ts(i, C)], rec[:], None,
                                ALU.mult)
        first = i == 0
        for k in range(4):
            for p in range(2):
                blk = 2 * k + p
                nc.tensor.matmul(accs[k][p * 64:(p + 1) * 64, :], vs[:],
                                 slab[:, ts(blk, 512)], start=first,
                                 stop=False, skip_group_check=True)
        for x in end_extras.get(i, ()):
            x()

    # close each accumulator with the 0.5*x_low residual: one matmul per
    # (k, p) against the 0.5*I stationary, reusing the zero-padded xlowT.
    # (0.5: the AllReduce over the core pair sums two copies of the residual
    # and bias-row terms, so each core contributes half.)
    for k in range(4):
        for p in range(2):
            blk = 2 * k + p
            nc.tensor.matmul(accs[k][p * 64:(p + 1) * 64, :], halfid,
                             xlowT[:, ts(blk, 512)], start=False,
                             stop=True, skip_group_check=True)

    # ---- tail: per-block evict e, pipelined with the odd-half partition-
    # move DMAs (fine-split: descriptor dispatch dominates, so 16-desc
    # pieces across many queues), transposes and the G matmuls ----
    e_h = big.tile([128, 4 * 512], DT_G, tag="e_h")
    e_h2 = big.tile([C, 4 * 512], DT_G, tag="e_h2")
    for k in range(4):
        nc.vector.tensor_copy(e_h[:, ts(k, 512)], accs[k][:])
        for lo, hi in ((0, 16), (16, 32), (32, 48), (48, 64)):
            nc.sync.dma_start(e_h2[lo:hi, ts(k, 512)],
                              e_h[64 + lo:64 + hi, ts(k, 512)])

    if TRUNC == 3:
        return finish(e_h[0:64, :])

    # groups ordered so even blocks (straight from e_h) go first while the
    # e_h2 partition-move DMAs complete.
    e_sp = big.tile([128, 32 * C], DT_G, tag="e_sp")
    gps = acc.tile([128, 512], dt.float32, tag="acc0")
    G = gps[0:64, 0:64]
    groups = [(0, 2), (4, 6), (1, 3), (5, 7)]
    n_done = 0
    for gi, blocks in enumerate(groups):
        pt = mm.tile([128, 512], DT_G, tag="mmt")
        tids = []
        for q, blk in enumerate([b for b in blocks for _ in range(4)]):
            sl = q % 4                 # 128-col slice within block
            t_idx = blk * 4 + sl
            tids.append(t_idx)
            k, p = blk // 2, blk % 2
            src = e_h if p == 0 else e_h2
            nc.tensor.transpose(
                pt[:, q * 64:(q + 1) * 64],
                src[0:64, k * 512 + sl * 128:k * 512 + (sl + 1) * 128],
                idf32)
        for half in range(2):
            t0 = tids[half * 4]
            nc.vector.tensor_copy(e_sp[:, t0 * 64:(t0 + 4) * 64],
                                  pt[:, half * 256:(half + 1) * 256])
        for t_idx in tids:
            nc.tensor.matmul(G, e_sp[:, ts(t_idx, C)], xl2sp[:, ts(t_idx, C)],
                             start=(n_done == 0), stop=(n_done == 31),
                             skip_group_check=True)
            n_done += 1

    if TRUNC == 36:
        gtmp = big.tile([C, C], dt.float32, tag="gtmp")
        nc.vector.tensor_copy(gtmp[:], G)
        return finish(gtmp[:])

    if TRUNC == 4:
        nc.vector.tensor_copy(gs_stage[0:64, :], G)
        return finish(gs_stage[0:64, :])

    # ---- AllReduce G over core pairs ----
    gs_red = big.tile([65, C], dt.float32, tag="gs_red")
    if USE_COLLECTIVE:
        g_in = dram.tile([65, C], dt.float32, tag="g_in")
        g_out = dram.tile([65, C], dt.float32, tag="g_out")
        # partition-split DMAs on parallel queues (descriptor dispatch
        # dominates)
        nc.vector.tensor_copy(gs_stage[0:64, :], G)
        for lo, hi in ((0, 16), (16, 32), (32, 48), (48, 65)):
            nc.sync.dma_start(g_in[lo:hi, :], gs_stage[lo:hi, :])
        nc.gpsimd.collective_compute(
            "AllReduce", ALU.add,
            ins=[g_in.opt()], outs=[g_out.opt()],
            replica_groups=[[0, 1], [2, 3], [4, 5], [6, 7]],
        )
        for lo, hi in ((0, 17), (17, 33), (33, 49), (49, 65)):
            nc.sync.dma_start(gs_red[lo:hi, :], g_out[lo:hi, :])
    else:
        nc.vector.tensor_copy(gs_red[:], gs_stage[:])

    # ---- t = We @ G + be x s  (fp32 matmuls) ----
    tps = mm.tile([128, 512], dt.float32, tag="mmt")
    t_ps = tps[0:64, 0:64]
    nc.tensor.matmul(t_ps, weT, gs_red[0:64, :], start=True, stop=False,
                     skip_group_check=True)
    nc.tensor.matmul(t_ps, be_row, gs_red[64:65, :], start=False,
                     stop=True, tile_position=(64, 0), skip_group_check=True)
    t_s = big.tile([C, C], dt.float32, tag="t_s")
    nc.vector.tensor_copy(t_s[:], t_ps)

    # ---- softmax over c: transpose -> [d, c], exp w/ max, normalize ----
    tt_ps = mm.tile([128, 512], dt.float32, tag="mmt")
    nc.tensor.transpose(tt_ps[0:64, 0:64], t_s[:], idf32)
    tmax = big.tile([C, 1], dt.float32, tag="tmax")
    nc.vector.reduce_max(tmax[:], tt_ps[0:64, 0:64], axis=mybir.AxisListType.X)
    nmax = big.tile([C, 1], dt.float32, tag="nmax")
    nc.vector.tensor_scalar(nmax[:], tmax[:], -1.0, None, ALU.mult)
    texp = big.tile([C, C], dt.float32, tag="texp")
    tsum = big.tile([C, 1], dt.float32, tag="tsum")
    nc.scalar.activation(texp[:], tt_ps[0:64, 0:64], AF.Exp, bias=nmax[:],
                         accum_out=tsum[:])
    trec = big.tile([C, 1], dt.float32, tag="trec")
    nc.vector.reciprocal(trec[:], tsum[:])

    # transpose unnormalized texp back -> [c, d] fp16; the 1/tsum[d]
    # normalization folds into the per-partition scale of the final
    # output evictions (out rows are d).
    tb_ps = mm.tile([128, 512], dt.float32, tag="mmt")
    nc.tensor.transpose(tb_ps[0:64, 0:64], texp[:], idf32)
    tsm = big.tile([128, C], DT_OUT, tag="tsm")
    nc.gpsimd.memset(tsm[C:128, :], 0.0)
    nc.vector.tensor_copy(tsm[0:C, :], tb_ps[0:64, 0:64])

    # ---- out^T[d, n-half] = tsm^T-pair @ xmidT  (K=128) ----
    # evictions alternate DVE/ACT; output DMA per 1024-col half overlaps
    osb = big.tile([C, MH], dt.float32, tag="osb")
    for k in range(4):
        op = mm.tile([C, 512], dt.float32, tag="mmt")
        nc.tensor.matmul(op[:], tsm[:], xmidT[:, ts(k, 512)],
                         start=True, stop=True)
        if k % 2 == 0:
            nc.vector.tensor_scalar(osb[:, ts(k, 512)], op[:], trec[:],
                                    None, ALU.mult)
        else:
            nc.scalar.activation(osb[:, ts(k, 512)], op[:], AF.Copy,
                                 scale=trec[:])
            for lo, hi in ((0, 16), (16, 32), (32, 48), (48, 64)):
                nc.sync.dma_start(
                    io["outp"][lo:hi, (k - 1) * 512:(k + 1) * 512],
                    osb[lo:hi, (k - 1) * 512:(k + 1) * 512])

    for p in (dram, acc, mm, slabp, big, const):
        p.release()


def _prep_inputs(x_latter, x, W, b):
    """Build the 8 per-core input maps from full inputs."""
    B = x_latter.shape[0]
    xr = x.reshape(B, C, HW).astype(np.float16)
    xlr = x_latter.reshape(B, C, HW).astype(np.float16)
    wT = {k: np.ascontiguousarray(W[k].T) for k in W}

    blob16 = np.zeros((128, 6 * C), np.float16)
    for i, n in enumerate(["low", "high", "value", "latter", "mid"]):
        blob16[0:C, i * C:(i + 1) * C] = wT[n].astype(np.float16)
    blob16[0:C, 5 * C:6 * C] = (0.5 * np.eye(C)).astype(np.float16)

    blob32 = np.zeros((128, 2 * C + 5), np.float32)
    blob32[0:C, 0:C] = wT["e_conv"]
    blob32[64, 0:C] = b["e_conv"].reshape(C)
    blob32[0:C, C:2 * C] = np.eye(C, dtype=np.float32)
    blob32[C:2 * C, C:2 * C] = np.eye(C, dtype=np.float32)
    for i, n in enumerate(["low", "high", "value", "latter", "mid"]):
        blob32[0:C, 2 * C + i] = b[n].reshape(C)

    maps = []
    for core in range(N_CORES):
        bi, h = core // 2, core % 2
        # roll columns so this core's own m-half sits at columns [0, MH)
        xin_c = np.roll(xr[bi], -h * MH, axis=1) if h else xr[bi]
        xlat_c = np.roll(xlr[bi], -h * MH, axis=1) if h else xlr[bi]
        maps.append({
            "xin": np.ascontiguousarray(xin_c),
            "xlat": np.ascontiguousarray(xlat_c),
            "blob16": blob16,
            "blob32": blob32,
        })
    return maps


def run(inputs, trace=False, trace_cores=None):
    if "nc" not in _CACHE:
        _CACHE["nc"] = build()
    nc = _CACHE["nc"]

    names = ["high", "low", "value", "e_conv", "mid", "latter"]
    W = {n: np.asarray(inputs[f"W_{n}"], dtype=np.float32) for n in names}
    b = {n: np.asarray(inputs[f"b_{n}"], dtype=np.float32).reshape(C, 1)
         for n in names}
    x = np.asarray(inputs["x"], dtype=np.float32)
    x_latter = np.asarray(inputs["x_latter"], dtype=np.float32)
    maps = _prep_inputs(x_latter, x, W, b)

    kw = {}
    if trace:
        kw = dict(trace=True,
                  trace_cores=trace_cores or list(range(N_CORES)))
    res = run_bass_kernel_spmd(nc, maps, core_ids=list(range(N_CORES)), **kw)

    B = x_latter.shape[0]
    out = np.empty((B, C, HW), dtype=np.float32)
    for core in range(N_CORES):
        bi, h = core // 2, core % 2
        out[bi][:, h * MH:(h + 1) * MH] = res.results[core]["outp"]
    H = int(np.sqrt(HW))
    return out.reshape(B, C, H, H), res


def kernel(**inputs):
    out, _ = run(inputs, trace=False)
    return out
